# revision 20
# baseline (speedup 1.0000x reference)
"""Conv5d (nn_Conv5d_36206574306083) Bass kernel for 8 trn2 NeuronCores.

Math: out[b,o,c,t] = (1/9) * sum_{i,j in 0..2} Conv3d_{ij}(x[b,:,c+i,t+j]) + mean_bias
with x [2,4,8,8,8,96,96], W [9,4,4,3,3,3], b [9,4].

Mapping: data-parallel over (b, c-group, t-group) -> 8 cores. Per core the
inner 5D conv is PSUM-accumulated banded matmuls:
  stationary S[(hslot4, ci4, d8) = 128, (o4, d8, hh2) = 64]
  moving rhs = pre-tiled x window [128, (c3, t3, w-range)]
Two h-pair chains run concurrently in the two PE column halves.

"wino" mode (default): F(2,3) Winograd along w. The DVE transforms each
x tile into 4 m-point tiles (d0-d2, d1+d2, d2-d1, d1-d3 over stride-2 w
pairs); rounds become (i, j, m) = 36 with half the streamed rows each
(216 vs 432); outputs are recovered as out0 = M0+M1+M2, out1 = M1-M2-M3
(the m=3 stationary is negated so both combines are adds).

Weight-stationary round-major ordering over groups of PSUM tiles plus an
IR post-pass that removes redundant LDWEIGHTS (the legalizer emits one
per matmul; reloading the identical stationary into the same column group
costs ~53ns of serial PE time each).
"""
import os
import sys

sys.path.insert(0, '/opt/trn_rl_repo')

import numpy as np

# ---------------------------------------------------------------- constants
B, C, CD, T, D, H, WD = 2, 4, 8, 8, 8, 96, 96
O = 4
CC, TT = CD - 2, T - 2          # 6, 6 output c/t positions
NCORES = 8
HB = 24                         # h blocks of 4 output rows
WH = 2                          # w halves of 48
NACC = 27                       # direct rounds: (i, j, kw)
NW = 36                         # wino rounds: (i, j, m)
KP = 128                        # contraction partitions (slot4, ci4, d8)
MP = 64                         # stationary cols (o4, d8, hh2)
FREE = 3 * 3 * 48               # 432 = (c3, t3, w48)
PFREE = 3 * 3 * 24              # 216 = (c3, t3, p24) per w half
XF = 5 * 5 * 98                 # 2450 free elems per x tile (c5, t5, w98)
XMF = 5 * 5 * 48                # 1200 free elems per transformed m tile

MODE = os.environ.get("CONV_MODE", "wino")  # "wino" | "direct"

_CACHE = {}


def _install_ntff_hook():
    """Optional: lets run_bass_kernel_spmd(trace=True) profile under axon."""
    import types
    name = 'antenv.axon_hooks'
    if name in sys.modules:
        return
    try:
        import antenv
        mod = types.ModuleType(name)
        mod._hook = None
        mod.set_axon_ntff_profile_hook = lambda h: setattr(mod, '_hook', h)
        mod.get_axon_ntff_profile_hook = lambda: mod._hook
        sys.modules[name] = mod
        antenv.axon_hooks = mod
        from trn_agent_boot.trn_boot import _ntff_profile_via_ctypes
        hook = _ntff_profile_via_ctypes('/opt/axon/libaxon_pjrt.so')
        if hook is not None:
            mod._hook = hook
    except Exception:
        pass


def _dedup_ldweights(nc, mybir):
    """Remove InstLdweights that reload the identical stationary into the
    same PE column group as the previous load for that group."""
    removed = 0
    for blk in nc.main_func.blocks:
        state = {}
        idx = 0
        while idx < len(blk.instructions):
            inst = blk.instructions[idx]
            if isinstance(inst, mybir.InstLdweights):
                tp = inst.tile_position
                key = (str(inst.ins[0]), str(tp), str(inst.tile_size))
                col = tp[1] if tp else 0
                si = inst.sync_info
                clean = si is None or (len(si.on_wait) == 0
                                       and len(si.on_update) == 0)
                if clean and state.get(col) == key:
                    del blk.instructions[idx]
                    removed += 1
                    continue
                state[col] = key
            idx += 1
    return removed


def _build_wino():
    import concourse.bacc as bacc
    import concourse.mybir as mybir
    from concourse.tile import TileContext
    from concourse.alu_op_type import AluOpType

    bf16 = mybir.dt.bfloat16
    f32 = mybir.dt.float32

    nc = bacc.Bacc("TRN2", target_bir_lowering=False, debug=False,
                   num_devices=NCORES)
    xs = nc.dram_tensor("xs", [HB, 2, KP, XF], bf16,
                        kind="ExternalInput").ap()
    stat = nc.dram_tensor("stat", [NW, KP, MP], bf16,
                          kind="ExternalInput").ap()
    bias = nc.dram_tensor("bias", [2 * MP, 1], f32, kind="ExternalInput").ap()
    out = nc.dram_tensor("out", [HB, WH, 2 * MP, 2 * PFREE], f32,
                         kind="ExternalOutput").ap()

    G = 2                       # hb per group; 2*G psum tiles of 2 banks each

    with TileContext(nc) as tc:
        with (tc.tile_pool(name="const", bufs=1) as cp,
              tc.tile_pool(name="xt", bufs=8) as xp,
              tc.tile_pool(name="Xt", bufs=8) as Xp,
              tc.tile_pool(name="ps", bufs=4, space="PSUM") as pp,
              tc.tile_pool(name="tmp", bufs=4) as tp,
              tc.tile_pool(name="ot", bufs=4) as op):
            st = cp.tile([KP, NW * MP], bf16)
            for a in range(NW):
                nc.sync.dma_start(out=st[:, a * MP:(a + 1) * MP], in_=stat[a])
            bt = cp.tile([2 * MP, 1], f32)
            nc.sync.dma_start(out=bt[:], in_=bias[:])

            def load_transform(hb, ab, hg):
                xt = xp.tile([KP, XF], bf16, tag="x", name=f"x{hg}{ab}")
                nc.sync.dma_start(out=xt[:], in_=xs[hb, ab])
                Xt = Xp.tile([KP, 4, XMF], bf16, tag="X", name=f"X{hg}{ab}")
                # host pre-split w into even/odd planes for unit-stride reads
                xv = xt[:].rearrange("k (c t e w2) -> k c t e w2",
                                     c=5, t=5, e=2)
                Xv = Xt[:].rearrange("k m (c t p) -> k m c t p", c=5, t=5)
                ev0 = xv[:, :, :, 0, 0:48]   # d0 = x[2p-1] (padded idx 2p)
                ev1 = xv[:, :, :, 0, 1:49]   # d2 = x[2p+1]
                od0 = xv[:, :, :, 1, 0:48]   # d1 = x[2p]
                od1 = xv[:, :, :, 1, 1:49]   # d3 = x[2p+2]
                nc.vector.tensor_sub(Xv[:, 0], ev0, ev1)   # m0 = d0 - d2
                nc.vector.tensor_add(Xv[:, 1], od0, ev1)   # m1 = d1 + d2
                nc.vector.tensor_sub(Xv[:, 2], ev1, od0)   # m2 = d2 - d1
                nc.gpsimd.tensor_sub(Xv[:, 3], od0, od1)   # m3 = d1 - d3
                return Xt

            def mm_set(ps, Xa, Xb, a, wh):
                ij, m = divmod(a, 4)
                i, j = divmod(ij, 3)
                sta = st[:, a * MP:(a + 1) * MP]
                rhs_a = Xa[:].rearrange("k m (c t p) -> k m c t p",
                                        c=5, t=5)[
                    :, m, i:i + 3, j:j + 3, wh * 24:wh * 24 + 24]
                rhs_b = Xb[:].rearrange("k m (c t p) -> k m c t p",
                                        c=5, t=5)[
                    :, m, i:i + 3, j:j + 3, wh * 24:wh * 24 + 24]
                # start=True clears has_written for the WHOLE bank, so only
                # the first matmul touching each bank (m=0 -> bank0,
                # m=2 -> bank1) may use it; the other m-region of the bank
                # starts from cleared bits (overwrite) without a new clear.
                nc.tensor.matmul(ps[0:MP, m, 0:PFREE], sta, rhs_a,
                                 start=(a in (0, 2)), stop=(ij == 8),
                                 skip_group_check=True)
                nc.tensor.matmul(ps[MP:2 * MP, m, 0:PFREE], sta, rhs_b,
                                 start=(a in (0, 2)), stop=(ij == 8),
                                 skip_group_check=True)

            def combine(ps, hb, wh):
                # DVE may read at most one PSUM operand per instruction;
                # the bias-seeded copies run on the idle scalar engine.
                tA = tp.tile([2 * MP, PFREE], f32, tag="tmp", name="tA")
                tB = tp.tile([2 * MP, PFREE], f32, tag="tmp", name="tB")
                ot = op.tile([2 * MP, 2, PFREE], f32, tag="ot", name="ot")
                # out0 = ((M0 + bias) + M1) + M2
                nc.scalar.activation(tA[:], ps[:, 0, 0:PFREE],
                                     mybir.ActivationFunctionType.Identity,
                                     bias=bt[:])
                nc.vector.tensor_add(tA[:], tA[:], ps[:, 1, 0:PFREE])
                nc.vector.tensor_add(ot[:, 0], tA[:], ps[:, 2, 0:PFREE])
                # out1 = ((-M3 + bias) + M1) - M2   (m=3 stationary negated)
                nc.scalar.activation(tB[:], ps[:, 3, 0:PFREE],
                                     mybir.ActivationFunctionType.Identity,
                                     bias=bt[:])
                nc.vector.tensor_add(tB[:], tB[:], ps[:, 1, 0:PFREE])
                nc.vector.tensor_sub(ot[:, 1], tB[:], ps[:, 2, 0:PFREE])
                nc.sync.dma_start(out=out[hb, wh], in_=ot[:])

            for g in range(HB // G):
                tiles = []
                for hg in range(G):
                    hb = g * G + hg
                    Xa = load_transform(hb, 0, hg)
                    Xb = load_transform(hb, 1, hg)
                    tiles.append((hb, Xa, Xb))
                pst = [[pp.tile([2 * MP, 4, 256], f32, tag="ps",
                                name=f"ps{hg}{wh}")
                        for wh in range(WH)] for hg in range(G)]
                if g == 0:
                    # set-major warmup: first matmuls only need the first
                    # x tile pair instead of the whole group
                    for hg, (hb, Xa, Xb) in enumerate(tiles):
                        for wh in range(WH):
                            for a in range(NW):
                                mm_set(pst[hg][wh], Xa, Xb, a, wh)
                            combine(pst[hg][wh], hb, wh)
                else:
                    for a in range(NW):
                        for hg, (hb, Xa, Xb) in enumerate(tiles):
                            for wh in range(WH):
                                mm_set(pst[hg][wh], Xa, Xb, a, wh)
                    for hg, (hb, Xa, Xb) in enumerate(tiles):
                        for wh in range(WH):
                            combine(pst[hg][wh], hb, wh)

    n_removed = _dedup_ldweights(nc, mybir)
    assert n_removed > 1000, f"ldweights dedup removed only {n_removed}"
    nc.compile()
    return nc


def _build_direct():
    import concourse.bacc as bacc
    import concourse.mybir as mybir
    from concourse.tile import TileContext

    bf16 = mybir.dt.bfloat16
    f32 = mybir.dt.float32

    nc = bacc.Bacc("TRN2", target_bir_lowering=False, debug=False,
                   num_devices=NCORES)
    xs = nc.dram_tensor("xs", [HB, 2, KP, XF], bf16,
                        kind="ExternalInput").ap()
    stat = nc.dram_tensor("stat", [NACC, KP, MP], bf16,
                          kind="ExternalInput").ap()
    bias = nc.dram_tensor("bias", [2 * MP, FREE], f32,
                          kind="ExternalInput").ap()
    out = nc.dram_tensor("out", [HB, WH, 2 * MP, FREE], f32,
                         kind="ExternalOutput").ap()

    ijkw = [(i, j, kw)
            for i in range(3) for j in range(3) for kw in range(3)]

    G = 3
    with TileContext(nc) as tc:
        with (tc.tile_pool(name="const", bufs=1) as cp,
              tc.tile_pool(name="xt", bufs=12) as xp,
              tc.tile_pool(name="ps", bufs=8, space="PSUM") as pp,
              tc.tile_pool(name="ot", bufs=4) as op):
            st = cp.tile([KP, NACC * MP], bf16)
            for a in range(NACC):
                nc.sync.dma_start(out=st[:, a * MP:(a + 1) * MP],
                                  in_=stat[a])
            bt = cp.tile([2 * MP, FREE], f32)
            nc.sync.dma_start(out=bt[:], in_=bias[:])

            for g in range(HB // G):
                tiles = []
                for hg in range(G):
                    hb = g * G + hg
                    ta = xp.tile([KP, XF], bf16, tag="x", name=f"xa{hg}")
                    tb = xp.tile([KP, XF], bf16, tag="x", name=f"xb{hg}")
                    nc.sync.dma_start(out=ta[:], in_=xs[hb, 0])
                    nc.sync.dma_start(out=tb[:], in_=xs[hb, 1])
                    tiles.append((hb, ta, tb))
                pst = [[pp.tile([2 * MP, FREE], f32, tag="ps",
                                name=f"ps{hg}{wh}")
                        for wh in range(WH)] for hg in range(G)]
                for a in range(NACC):
                    i, j, kw = ijkw[a]
                    sta = st[:, a * MP:(a + 1) * MP]
                    for hg, (hb, ta, tb) in enumerate(tiles):
                        for wh in range(WH):
                            base = wh * 48
                            rhs_a = ta[:].rearrange(
                                "k (c t w) -> k c t w", c=5, t=5)[
                                :, i:i + 3, j:j + 3,
                                base + kw:base + kw + 48]
                            rhs_b = tb[:].rearrange(
                                "k (c t w) -> k c t w", c=5, t=5)[
                                :, i:i + 3, j:j + 3,
                                base + kw:base + kw + 48]
                            ps = pst[hg][wh]
                            nc.tensor.matmul(
                                ps[0:MP, :], sta, rhs_a,
                                start=(a == 0), stop=(a == NACC - 1))
                            nc.tensor.matmul(
                                ps[MP:2 * MP, :], sta, rhs_b,
                                start=(a == 0), stop=(a == NACC - 1))
                for hg, (hb, ta, tb) in enumerate(tiles):
                    for wh in range(WH):
                        ot = op.tile([2 * MP, FREE], f32, tag="ot", name="ot")
                        nc.vector.tensor_add(ot[:], pst[hg][wh][:], bt[:])
                        nc.sync.dma_start(out=out[hb, wh], in_=ot[:])

    n_removed = _dedup_ldweights(nc, mybir)
    assert n_removed > 1000, f"ldweights dedup removed only {n_removed}"
    nc.compile()
    return nc


def _band_stat(vals):
    """Scatter per-(kd,kh) [ci,o] values into the banded stationary.
    vals[kd][kh] -> [ci, o]; returns [KP, MP] = [(slot,ci,dp), (o,d,hh)]."""
    S = np.zeros((4, C, D, O, D, 2), np.float32)
    for slot in range(4):
        for hh in range(2):
            kh = slot - hh
            if not 0 <= kh <= 2:
                continue
            for d in range(D):
                for kd in range(3):
                    dp = d + kd - 1
                    if not 0 <= dp < D:
                        continue
                    S[slot, :, dp, :, d, hh] = vals[kd][kh]
    return S.reshape(KP, MP)


def _host_prep(x, Wk, b, mode):
    import ml_dtypes
    npdt = ml_dtypes.bfloat16

    mean_b = (b.sum(0) / 9.0).astype(np.float32)

    if mode == "wino":
        Gm = np.array([[1, 0, 0], [.5, .5, .5], [.5, -.5, .5], [0, 0, 1]],
                      np.float64)
        S = np.zeros((NW, KP, MP), np.float32)
        for i in range(3):
            for j in range(3):
                ij = i * 3 + j
                # Gg[m, o, ci, kd, kh]
                Gg = np.einsum('mw,ockhw->mockh', Gm,
                               Wk[ij].astype(np.float64) / 9.0)
                Gg[3] = -Gg[3]   # negated so the combine is all adds
                for m in range(4):
                    vals = [[Gg[m, :, :, kd, kh].T.astype(np.float32)
                             for kh in range(3)] for kd in range(3)]
                    S[ij * 4 + m] = _band_stat(vals)
        S = S.astype(npdt)
        bias_arr = np.empty((2 * MP, 1), np.float32)
        for p in range(2 * MP):
            bias_arr[p, 0] = mean_b[(p % MP) // 16]
    else:
        S = np.zeros((NACC, KP, MP), np.float32)
        for i in range(3):
            for j in range(3):
                for kw in range(3):
                    a = (i * 3 + j) * 3 + kw
                    vals = [[Wk[i * 3 + j, :, :, kd, kh, kw].T / 9.0
                             for kh in range(3)] for kd in range(3)]
                    S[a] = _band_stat(vals)
        S = S.astype(npdt)
        bias_arr = np.empty((2 * MP, FREE), np.float32)
        for p in range(2 * MP):
            bias_arr[p, :] = mean_b[(p % MP) // 16]

    in_maps = []
    for core in range(NCORES):
        bb, cg, tg = core // 4, (core // 2) % 2, core % 2
        xsh = x[bb, :, cg * 3:cg * 3 + 5, tg * 3:tg * 3 + 5]  # [4,5,5,8,96,96]
        xpad = np.zeros((C, 5, 5, D, H + 2, WD + 2), np.float32)
        xpad[:, :, :, :, 1:H + 1, 1:WD + 1] = xsh
        # xs[hb, ab, (slot,ci,d), (cdim,td,w)]; window rows h'=4hb-1+2ab+slot
        xt = np.empty((HB, 2, 4, C, D, 5, 5, WD + 2), np.float32)
        for hb in range(HB):
            for ab in range(2):
                h0 = 4 * hb + 2 * ab
                blk = xpad[:, :, :, :, h0:h0 + 4, :]
                xt[hb, ab] = blk.transpose(4, 0, 3, 1, 2, 5)
        if mode == "wino":
            # split w into (even, odd) planes: [..., 98] -> [..., 2, 49]
            xt = np.ascontiguousarray(
                xt.reshape(HB, 2, 4, C, D, 5, 5, 49, 2).transpose(
                    0, 1, 2, 3, 4, 5, 6, 8, 7))
        in_maps.append({
            "xs": xt.reshape(HB, 2, KP, XF).astype(npdt),
            "stat": S,
            "bias": bias_arr,
        })
    return in_maps


def kernel(x, W, b, trace=False):
    x = np.asarray(x, np.float32)
    W = np.asarray(W, np.float32)
    b = np.asarray(b, np.float32)

    mode = MODE
    if mode not in _CACHE:
        _install_ntff_hook()
        _CACHE[mode] = _build_wino() if mode == "wino" else _build_direct()
    nc = _CACHE[mode]

    from concourse.bass_utils import run_bass_kernel_spmd
    in_maps = _host_prep(x, W, b, mode)
    res = run_bass_kernel_spmd(nc, in_maps, core_ids=list(range(NCORES)),
                               trace=trace)
    kernel.last_exec_ns = res.exec_time_ns

    outf = np.empty((B, O, CC, TT, D, H, WD), np.float32)
    for core in range(NCORES):
        bb, cg, tg = core // 4, (core // 2) % 2, core % 2
        r = res.results[core]["out"]  # [HB, WH, 128, 432]
        if mode == "wino":
            r = r.reshape(HB, WH, 2, O, D, 2, 2, 3, 3, 24)
            # (hb, wh, hp, o, d, hh, e, c, t, p)
            #  -> (o, c, t, d, hb, hp, hh, wh, p, e)
            r = r.transpose(3, 7, 8, 4, 0, 2, 5, 1, 9, 6)
            r = r.reshape(O, 3, 3, D, H, WD)
        else:
            r = r.reshape(HB, WH, 2, O, D, 2, 3, 3, 48)
            # (hb, wh, hp, o, d, hh, c, t, wc)
            r = r.transpose(3, 6, 7, 4, 0, 2, 5, 1, 8)
            r = r.reshape(O, 3, 3, D, H, WD)
        outf[bb, :, cg * 3:cg * 3 + 3, tg * 3:tg * 3 + 3] = r
    return outf


kernel.last_exec_ns = None


# revision 22
# speedup vs baseline: 1.5270x; 1.5270x over previous
"""Conv5d (nn_Conv5d_36206574306083) Bass kernel for 8 trn2 NeuronCores.

Math: out[b,o,c,t] = (1/9) * sum_{i,j in 0..2} Conv3d_{ij}(x[b,:,c+i,t+j]) + mean_bias
with x [2,4,8,8,8,96,96], W [9,4,4,3,3,3], b [9,4].

Mapping: data-parallel over (b, c-group, t-group) -> 8 cores. Per core the
inner 5D conv is PSUM-accumulated banded matmuls:
  stationary S[(hslot4, ci4, d8) = 128, (o4, d8, hh2) = 64]
  moving rhs = pre-tiled x window [128, (c3, t3, w-range)]
Two h-pair chains run concurrently in the two PE column halves.

"wino" mode (default): F(2,3) Winograd along w. The DVE transforms each
x tile into 4 m-point tiles (d0-d2, d1+d2, d2-d1, d1-d3 over stride-2 w
pairs); rounds become (i, j, m) = 36 with half the streamed rows each
(216 vs 432); outputs are recovered as out0 = M0+M1+M2, out1 = M1-M2-M3
(the m=3 stationary is negated so both combines are adds).

Weight-stationary round-major ordering over groups of PSUM tiles plus an
IR post-pass that removes redundant LDWEIGHTS (the legalizer emits one
per matmul; reloading the identical stationary into the same column group
costs ~53ns of serial PE time each).
"""
import os
import sys

sys.path.insert(0, '/opt/trn_rl_repo')

import numpy as np

# ---------------------------------------------------------------- constants
B, C, CD, T, D, H, WD = 2, 4, 8, 8, 8, 96, 96
O = 4
CC, TT = CD - 2, T - 2          # 6, 6 output c/t positions
NCORES = 8
HB = 24                         # h blocks of 4 output rows
WH = 2                          # w halves of 48
NACC = 27                       # direct rounds: (i, j, kw)
NW = 36                         # wino rounds: (i, j, m)
KP = 128                        # contraction partitions (slot4, ci4, d8)
MP = 64                         # stationary cols (o4, d8, hh2)
FREE = 3 * 3 * 48               # 432 = (c3, t3, w48)
PFREE = 3 * 3 * 24              # 216 = (c3, t3, p24) per w half
XF = 5 * 5 * 98                 # 2450 free elems per x tile (c5, t5, w98)
XMF = 5 * 5 * 48                # 1200 free elems per transformed m tile

MODE = os.environ.get("CONV_MODE", "direct")  # "direct" | "wino"

_CACHE = {}


def _install_ntff_hook():
    """Optional: lets run_bass_kernel_spmd(trace=True) profile under axon."""
    import types
    name = 'antenv.axon_hooks'
    if name in sys.modules:
        return
    try:
        import antenv
        mod = types.ModuleType(name)
        mod._hook = None
        mod.set_axon_ntff_profile_hook = lambda h: setattr(mod, '_hook', h)
        mod.get_axon_ntff_profile_hook = lambda: mod._hook
        sys.modules[name] = mod
        antenv.axon_hooks = mod
        from trn_agent_boot.trn_boot import _ntff_profile_via_ctypes
        hook = _ntff_profile_via_ctypes('/opt/axon/libaxon_pjrt.so')
        if hook is not None:
            mod._hook = hook
    except Exception:
        pass


def _dedup_ldweights(nc, mybir):
    """Remove InstLdweights that reload the identical stationary into the
    same PE column group as the previous load for that group."""
    removed = 0
    for blk in nc.main_func.blocks:
        state = {}
        idx = 0
        while idx < len(blk.instructions):
            inst = blk.instructions[idx]
            if isinstance(inst, mybir.InstLdweights):
                tp = inst.tile_position
                key = (str(inst.ins[0]), str(tp), str(inst.tile_size))
                col = tp[1] if tp else 0
                si = inst.sync_info
                clean = si is None or (len(si.on_wait) == 0
                                       and len(si.on_update) == 0)
                if clean and state.get(col) == key:
                    del blk.instructions[idx]
                    removed += 1
                    continue
                state[col] = key
            idx += 1
    return removed


def _build_wino():
    import concourse.bacc as bacc
    import concourse.mybir as mybir
    from concourse.tile import TileContext
    from concourse.alu_op_type import AluOpType

    bf16 = mybir.dt.bfloat16
    f32 = mybir.dt.float32

    nc = bacc.Bacc("TRN2", target_bir_lowering=False, debug=False,
                   num_devices=NCORES)
    xs = nc.dram_tensor("xs", [HB, 2, KP, XF], bf16,
                        kind="ExternalInput").ap()
    stat = nc.dram_tensor("stat", [NW, KP, MP], bf16,
                          kind="ExternalInput").ap()
    bias = nc.dram_tensor("bias", [2 * MP, 1], f32, kind="ExternalInput").ap()
    out = nc.dram_tensor("out", [HB, WH, 2 * MP, 2 * PFREE], f32,
                         kind="ExternalOutput").ap()

    G = 2                       # hb per group; 2*G psum tiles of 2 banks each

    with TileContext(nc) as tc:
        with (tc.tile_pool(name="const", bufs=1) as cp,
              tc.tile_pool(name="xt", bufs=8) as xp,
              tc.tile_pool(name="Xt", bufs=8) as Xp,
              tc.tile_pool(name="ps", bufs=4, space="PSUM") as pp,
              tc.tile_pool(name="tmp", bufs=4) as tp,
              tc.tile_pool(name="ot", bufs=4) as op):
            st = cp.tile([KP, NW * MP], bf16)
            for a in range(NW):
                nc.sync.dma_start(out=st[:, a * MP:(a + 1) * MP], in_=stat[a])
            bt = cp.tile([2 * MP, 1], f32)
            nc.sync.dma_start(out=bt[:], in_=bias[:])

            def load_transform(hb, ab, hg):
                xt = xp.tile([KP, XF], bf16, tag="x", name=f"x{hg}{ab}")
                nc.sync.dma_start(out=xt[:], in_=xs[hb, ab])
                Xt = Xp.tile([KP, 4, XMF], bf16, tag="X", name=f"X{hg}{ab}")
                # host pre-split w into even/odd planes for unit-stride reads
                xv = xt[:].rearrange("k (c t e w2) -> k c t e w2",
                                     c=5, t=5, e=2)
                Xv = Xt[:].rearrange("k m (c t p) -> k m c t p", c=5, t=5)
                ev0 = xv[:, :, :, 0, 0:48]   # d0 = x[2p-1] (padded idx 2p)
                ev1 = xv[:, :, :, 0, 1:49]   # d2 = x[2p+1]
                od0 = xv[:, :, :, 1, 0:48]   # d1 = x[2p]
                od1 = xv[:, :, :, 1, 1:49]   # d3 = x[2p+2]
                nc.vector.tensor_sub(Xv[:, 0], ev0, ev1)   # m0 = d0 - d2
                nc.vector.tensor_add(Xv[:, 1], od0, ev1)   # m1 = d1 + d2
                nc.vector.tensor_sub(Xv[:, 2], ev1, od0)   # m2 = d2 - d1
                nc.gpsimd.tensor_sub(Xv[:, 3], od0, od1)   # m3 = d1 - d3
                return Xt

            def mm_set(ps, Xa, Xb, a, wh):
                ij, m = divmod(a, 4)
                i, j = divmod(ij, 3)
                sta = st[:, a * MP:(a + 1) * MP]
                rhs_a = Xa[:].rearrange("k m (c t p) -> k m c t p",
                                        c=5, t=5)[
                    :, m, i:i + 3, j:j + 3, wh * 24:wh * 24 + 24]
                rhs_b = Xb[:].rearrange("k m (c t p) -> k m c t p",
                                        c=5, t=5)[
                    :, m, i:i + 3, j:j + 3, wh * 24:wh * 24 + 24]
                # start=True clears has_written for the WHOLE bank, so only
                # the first matmul touching each bank (m=0 -> bank0,
                # m=2 -> bank1) may use it; the other m-region of the bank
                # starts from cleared bits (overwrite) without a new clear.
                nc.tensor.matmul(ps[0:MP, m, 0:PFREE], sta, rhs_a,
                                 start=(a in (0, 2)), stop=(ij == 8),
                                 skip_group_check=True)
                nc.tensor.matmul(ps[MP:2 * MP, m, 0:PFREE], sta, rhs_b,
                                 start=(a in (0, 2)), stop=(ij == 8),
                                 skip_group_check=True)

            def combine(ps, hb, wh):
                # DVE may read at most one PSUM operand per instruction;
                # the bias-seeded copies run on the idle scalar engine.
                tA = tp.tile([2 * MP, PFREE], f32, tag="tmp", name="tA")
                tB = tp.tile([2 * MP, PFREE], f32, tag="tmp", name="tB")
                ot = op.tile([2 * MP, 2, PFREE], f32, tag="ot", name="ot")
                # out0 = ((M0 + bias) + M1) + M2
                nc.scalar.activation(tA[:], ps[:, 0, 0:PFREE],
                                     mybir.ActivationFunctionType.Identity,
                                     bias=bt[:])
                nc.vector.tensor_add(tA[:], tA[:], ps[:, 1, 0:PFREE])
                nc.vector.tensor_add(ot[:, 0], tA[:], ps[:, 2, 0:PFREE])
                # out1 = ((-M3 + bias) + M1) - M2   (m=3 stationary negated)
                nc.scalar.activation(tB[:], ps[:, 3, 0:PFREE],
                                     mybir.ActivationFunctionType.Identity,
                                     bias=bt[:])
                nc.vector.tensor_add(tB[:], tB[:], ps[:, 1, 0:PFREE])
                nc.vector.tensor_sub(ot[:, 1], tB[:], ps[:, 2, 0:PFREE])
                nc.sync.dma_start(out=out[hb, wh], in_=ot[:])

            for g in range(HB // G):
                tiles = []
                for hg in range(G):
                    hb = g * G + hg
                    Xa = load_transform(hb, 0, hg)
                    Xb = load_transform(hb, 1, hg)
                    tiles.append((hb, Xa, Xb))
                pst = [[pp.tile([2 * MP, 4, 256], f32, tag="ps",
                                name=f"ps{hg}{wh}")
                        for wh in range(WH)] for hg in range(G)]
                if g == 0:
                    # set-major warmup: first matmuls only need the first
                    # x tile pair instead of the whole group
                    for hg, (hb, Xa, Xb) in enumerate(tiles):
                        for wh in range(WH):
                            for a in range(NW):
                                mm_set(pst[hg][wh], Xa, Xb, a, wh)
                            combine(pst[hg][wh], hb, wh)
                else:
                    for a in range(NW):
                        for hg, (hb, Xa, Xb) in enumerate(tiles):
                            for wh in range(WH):
                                mm_set(pst[hg][wh], Xa, Xb, a, wh)
                    for hg, (hb, Xa, Xb) in enumerate(tiles):
                        for wh in range(WH):
                            combine(pst[hg][wh], hb, wh)

    n_removed = _dedup_ldweights(nc, mybir)
    assert n_removed > 1000, f"ldweights dedup removed only {n_removed}"
    nc.compile()
    return nc


def _build_direct():
    import concourse.bacc as bacc
    import concourse.mybir as mybir
    from concourse.tile import TileContext

    bf16 = mybir.dt.bfloat16
    f32 = mybir.dt.float32

    nc = bacc.Bacc("TRN2", target_bir_lowering=False, debug=False,
                   num_devices=NCORES)
    xs = nc.dram_tensor("xs", [HB, 2, KP, XF], bf16,
                        kind="ExternalInput").ap()
    stat = nc.dram_tensor("stat", [NACC, KP, MP], bf16,
                          kind="ExternalInput").ap()
    bias = nc.dram_tensor("bias", [2 * MP, FREE], f32,
                          kind="ExternalInput").ap()
    out = nc.dram_tensor("out", [HB, WH, 2 * MP, FREE], f32,
                         kind="ExternalOutput").ap()

    ijkw = [(i, j, kw)
            for i in range(3) for j in range(3) for kw in range(3)]

    G = 3
    with TileContext(nc) as tc:
        with (tc.tile_pool(name="const", bufs=1) as cp,
              tc.tile_pool(name="xt", bufs=12) as xp,
              tc.tile_pool(name="ps", bufs=8, space="PSUM") as pp,
              tc.tile_pool(name="ot", bufs=4) as op):
            st = cp.tile([KP, NACC * MP], bf16)
            for a in range(NACC):
                nc.sync.dma_start(out=st[:, a * MP:(a + 1) * MP],
                                  in_=stat[a])
            bt = cp.tile([2 * MP, FREE], f32)
            nc.sync.dma_start(out=bt[:], in_=bias[:])

            for g in range(HB // G):
                tiles = []
                for hg in range(G):
                    hb = g * G + hg
                    ta = xp.tile([KP, XF], bf16, tag="x", name=f"xa{hg}")
                    tb = xp.tile([KP, XF], bf16, tag="x", name=f"xb{hg}")
                    nc.sync.dma_start(out=ta[:], in_=xs[hb, 0])
                    nc.sync.dma_start(out=tb[:], in_=xs[hb, 1])
                    tiles.append((hb, ta, tb))
                pst = [[pp.tile([2 * MP, FREE], f32, tag="ps",
                                name=f"ps{hg}{wh}")
                        for wh in range(WH)] for hg in range(G)]

                def mm_set(ps, ta, tb, a, wh):
                    i, j, kw = ijkw[a]
                    sta = st[:, a * MP:(a + 1) * MP]
                    base = wh * 48
                    rhs_a = ta[:].rearrange(
                        "k (c t w) -> k c t w", c=5, t=5)[
                        :, i:i + 3, j:j + 3, base + kw:base + kw + 48]
                    rhs_b = tb[:].rearrange(
                        "k (c t w) -> k c t w", c=5, t=5)[
                        :, i:i + 3, j:j + 3, base + kw:base + kw + 48]
                    nc.tensor.matmul(ps[0:MP, :], sta, rhs_a,
                                     start=(a == 0), stop=(a == NACC - 1))
                    nc.tensor.matmul(ps[MP:2 * MP, :], sta, rhs_b,
                                     start=(a == 0), stop=(a == NACC - 1))

                def drain(ps, hb, wh):
                    ot = op.tile([2 * MP, FREE], f32, tag="ot", name="ot")
                    nc.vector.tensor_add(ot[:], ps[:], bt[:])
                    nc.sync.dma_start(out=out[hb, wh], in_=ot[:])

                if g == 0:
                    # set-major warmup: first matmuls only need the first
                    # x tile pair instead of the whole group's DMAs
                    for hg, (hb, ta, tb) in enumerate(tiles):
                        for wh in range(WH):
                            for a in range(NACC):
                                mm_set(pst[hg][wh], ta, tb, a, wh)
                            drain(pst[hg][wh], hb, wh)
                else:
                    for a in range(NACC):
                        for hg, (hb, ta, tb) in enumerate(tiles):
                            for wh in range(WH):
                                mm_set(pst[hg][wh], ta, tb, a, wh)
                    for hg, (hb, ta, tb) in enumerate(tiles):
                        for wh in range(WH):
                            drain(pst[hg][wh], hb, wh)

    n_removed = _dedup_ldweights(nc, mybir)
    assert n_removed > 1000, f"ldweights dedup removed only {n_removed}"
    nc.compile()
    return nc


def _band_stat(vals):
    """Scatter per-(kd,kh) [ci,o] values into the banded stationary.
    vals[kd][kh] -> [ci, o]; returns [KP, MP] = [(slot,ci,dp), (o,d,hh)]."""
    S = np.zeros((4, C, D, O, D, 2), np.float32)
    for slot in range(4):
        for hh in range(2):
            kh = slot - hh
            if not 0 <= kh <= 2:
                continue
            for d in range(D):
                for kd in range(3):
                    dp = d + kd - 1
                    if not 0 <= dp < D:
                        continue
                    S[slot, :, dp, :, d, hh] = vals[kd][kh]
    return S.reshape(KP, MP)


def _host_prep(x, Wk, b, mode):
    import ml_dtypes
    npdt = ml_dtypes.bfloat16

    mean_b = (b.sum(0) / 9.0).astype(np.float32)

    if mode == "wino":
        Gm = np.array([[1, 0, 0], [.5, .5, .5], [.5, -.5, .5], [0, 0, 1]],
                      np.float64)
        S = np.zeros((NW, KP, MP), np.float32)
        for i in range(3):
            for j in range(3):
                ij = i * 3 + j
                # Gg[m, o, ci, kd, kh]
                Gg = np.einsum('mw,ockhw->mockh', Gm,
                               Wk[ij].astype(np.float64) / 9.0)
                Gg[3] = -Gg[3]   # negated so the combine is all adds
                for m in range(4):
                    vals = [[Gg[m, :, :, kd, kh].T.astype(np.float32)
                             for kh in range(3)] for kd in range(3)]
                    S[ij * 4 + m] = _band_stat(vals)
        S = S.astype(npdt)
        bias_arr = np.empty((2 * MP, 1), np.float32)
        for p in range(2 * MP):
            bias_arr[p, 0] = mean_b[(p % MP) // 16]
    else:
        S = np.zeros((NACC, KP, MP), np.float32)
        for i in range(3):
            for j in range(3):
                for kw in range(3):
                    a = (i * 3 + j) * 3 + kw
                    vals = [[Wk[i * 3 + j, :, :, kd, kh, kw].T / 9.0
                             for kh in range(3)] for kd in range(3)]
                    S[a] = _band_stat(vals)
        S = S.astype(npdt)
        bias_arr = np.empty((2 * MP, FREE), np.float32)
        for p in range(2 * MP):
            bias_arr[p, :] = mean_b[(p % MP) // 16]

    in_maps = []
    for core in range(NCORES):
        bb, cg, tg = core // 4, (core // 2) % 2, core % 2
        xsh = x[bb, :, cg * 3:cg * 3 + 5, tg * 3:tg * 3 + 5]  # [4,5,5,8,96,96]
        xpad = np.zeros((C, 5, 5, D, H + 2, WD + 2), np.float32)
        xpad[:, :, :, :, 1:H + 1, 1:WD + 1] = xsh
        # xs[hb, ab, (slot,ci,d), (cdim,td,w)]; window rows h'=4hb-1+2ab+slot
        xt = np.empty((HB, 2, 4, C, D, 5, 5, WD + 2), np.float32)
        for hb in range(HB):
            for ab in range(2):
                h0 = 4 * hb + 2 * ab
                blk = xpad[:, :, :, :, h0:h0 + 4, :]
                xt[hb, ab] = blk.transpose(4, 0, 3, 1, 2, 5)
        if mode == "wino":
            # split w into (even, odd) planes: [..., 98] -> [..., 2, 49]
            xt = np.ascontiguousarray(
                xt.reshape(HB, 2, 4, C, D, 5, 5, 49, 2).transpose(
                    0, 1, 2, 3, 4, 5, 6, 8, 7))
        in_maps.append({
            "xs": xt.reshape(HB, 2, KP, XF).astype(npdt),
            "stat": S,
            "bias": bias_arr,
        })
    return in_maps


def kernel(x, W, b, trace=False):
    x = np.asarray(x, np.float32)
    W = np.asarray(W, np.float32)
    b = np.asarray(b, np.float32)

    mode = MODE
    if mode not in _CACHE:
        _install_ntff_hook()
        _CACHE[mode] = _build_wino() if mode == "wino" else _build_direct()
    nc = _CACHE[mode]

    from concourse.bass_utils import run_bass_kernel_spmd
    in_maps = _host_prep(x, W, b, mode)
    res = run_bass_kernel_spmd(nc, in_maps, core_ids=list(range(NCORES)),
                               trace=trace)
    kernel.last_exec_ns = res.exec_time_ns

    outf = np.empty((B, O, CC, TT, D, H, WD), np.float32)
    for core in range(NCORES):
        bb, cg, tg = core // 4, (core // 2) % 2, core % 2
        r = res.results[core]["out"]  # [HB, WH, 128, 432]
        if mode == "wino":
            r = r.reshape(HB, WH, 2, O, D, 2, 2, 3, 3, 24)
            # (hb, wh, hp, o, d, hh, e, c, t, p)
            #  -> (o, c, t, d, hb, hp, hh, wh, p, e)
            r = r.transpose(3, 7, 8, 4, 0, 2, 5, 1, 9, 6)
            r = r.reshape(O, 3, 3, D, H, WD)
        else:
            r = r.reshape(HB, WH, 2, O, D, 2, 3, 3, 48)
            # (hb, wh, hp, o, d, hh, c, t, wc)
            r = r.transpose(3, 6, 7, 4, 0, 2, 5, 1, 8)
            r = r.reshape(O, 3, 3, D, H, WD)
        outf[bb, :, cg * 3:cg * 3 + 3, tg * 3:tg * 3 + 3] = r
    return outf


kernel.last_exec_ns = None


# revision 24
# speedup vs baseline: 1.6404x; 1.0742x over previous
"""Conv5d (nn_Conv5d_36206574306083) Bass kernel for 8 trn2 NeuronCores.

Math: out[b,o,c,t] = (1/9) * sum_{i,j in 0..2} Conv3d_{ij}(x[b,:,c+i,t+j]) + mean_bias
with x [2,4,8,8,8,96,96], W [9,4,4,3,3,3], b [9,4].

Mapping: data-parallel over (b, c-group, t-group) -> 8 cores. Per core the
inner 5D conv is PSUM-accumulated banded matmuls:
  stationary S[(hslot4, ci4, d8) = 128, (o4, d8, hh2) = 64]
  moving rhs = pre-tiled x window [128, (c3, t3, w-range)]
Two h-pair chains run concurrently in the two PE column halves.

"wino" mode (default): F(2,3) Winograd along w. The DVE transforms each
x tile into 4 m-point tiles (d0-d2, d1+d2, d2-d1, d1-d3 over stride-2 w
pairs); rounds become (i, j, m) = 36 with half the streamed rows each
(216 vs 432); outputs are recovered as out0 = M0+M1+M2, out1 = M1-M2-M3
(the m=3 stationary is negated so both combines are adds).

Weight-stationary round-major ordering over groups of PSUM tiles plus an
IR post-pass that removes redundant LDWEIGHTS (the legalizer emits one
per matmul; reloading the identical stationary into the same column group
costs ~53ns of serial PE time each).
"""
import os
import sys

sys.path.insert(0, '/opt/trn_rl_repo')

import numpy as np

# ---------------------------------------------------------------- constants
B, C, CD, T, D, H, WD = 2, 4, 8, 8, 8, 96, 96
O = 4
CC, TT = CD - 2, T - 2          # 6, 6 output c/t positions
NCORES = 8
HB = 24                         # h blocks of 4 output rows
WH = 2                          # w halves of 48
NACC = 27                       # direct rounds: (i, j, kw)
NW = 36                         # wino rounds: (i, j, m)
KP = 128                        # contraction partitions (slot4, ci4, d8)
MP = 64                         # stationary cols (o4, d8, hh2)
FREE = 3 * 3 * 48               # 432 = (c3, t3, w48)
PFREE = 3 * 3 * 24              # 216 = (c3, t3, p24) per w half
XF = 5 * 5 * 98                 # 2450 free elems per x tile (c5, t5, w98)
XMF = 5 * 5 * 48                # 1200 free elems per transformed m tile

MODE = os.environ.get("CONV_MODE", "direct")  # "direct" | "wino"

_CACHE = {}


def _install_ntff_hook():
    """Optional: lets run_bass_kernel_spmd(trace=True) profile under axon."""
    import types
    name = 'antenv.axon_hooks'
    if name in sys.modules:
        return
    try:
        import antenv
        mod = types.ModuleType(name)
        mod._hook = None
        mod.set_axon_ntff_profile_hook = lambda h: setattr(mod, '_hook', h)
        mod.get_axon_ntff_profile_hook = lambda: mod._hook
        sys.modules[name] = mod
        antenv.axon_hooks = mod
        from trn_agent_boot.trn_boot import _ntff_profile_via_ctypes
        hook = _ntff_profile_via_ctypes('/opt/axon/libaxon_pjrt.so')
        if hook is not None:
            mod._hook = hook
    except Exception:
        pass


def _dedup_ldweights(nc, mybir):
    """Remove InstLdweights that reload the identical stationary into the
    same PE column group as the previous load for that group."""
    removed = 0
    for blk in nc.main_func.blocks:
        state = {}
        idx = 0
        while idx < len(blk.instructions):
            inst = blk.instructions[idx]
            if isinstance(inst, mybir.InstLdweights):
                tp = inst.tile_position
                key = (str(inst.ins[0]), str(tp), str(inst.tile_size))
                col = tp[1] if tp else 0
                si = inst.sync_info
                clean = si is None or (len(si.on_wait) == 0
                                       and len(si.on_update) == 0)
                if clean and state.get(col) == key:
                    del blk.instructions[idx]
                    removed += 1
                    continue
                state[col] = key
            idx += 1
    return removed


def _build_wino():
    import concourse.bacc as bacc
    import concourse.mybir as mybir
    from concourse.tile import TileContext
    from concourse.alu_op_type import AluOpType

    bf16 = mybir.dt.bfloat16
    f32 = mybir.dt.float32

    nc = bacc.Bacc("TRN2", target_bir_lowering=False, debug=False,
                   num_devices=NCORES)
    xs = nc.dram_tensor("xs", [HB, 2, KP, XF], bf16,
                        kind="ExternalInput").ap()
    stat = nc.dram_tensor("stat", [NW, KP, MP], bf16,
                          kind="ExternalInput").ap()
    bias = nc.dram_tensor("bias", [2 * MP, 1], f32, kind="ExternalInput").ap()
    out = nc.dram_tensor("out", [HB, WH, 2 * MP, 2 * PFREE], f32,
                         kind="ExternalOutput").ap()

    G = 2                       # hb per group; 2*G psum tiles of 2 banks each

    with TileContext(nc) as tc:
        with (tc.tile_pool(name="const", bufs=1) as cp,
              tc.tile_pool(name="xt", bufs=8) as xp,
              tc.tile_pool(name="Xt", bufs=8) as Xp,
              tc.tile_pool(name="ps", bufs=4, space="PSUM") as pp,
              tc.tile_pool(name="tmp", bufs=4) as tp,
              tc.tile_pool(name="ot", bufs=4) as op):
            st = cp.tile([KP, NW * MP], bf16)
            for a in range(NW):
                nc.sync.dma_start(out=st[:, a * MP:(a + 1) * MP], in_=stat[a])
            bt = cp.tile([2 * MP, 1], f32)
            nc.sync.dma_start(out=bt[:], in_=bias[:])

            def load_transform(hb, ab, hg):
                xt = xp.tile([KP, XF], bf16, tag="x", name=f"x{hg}{ab}")
                nc.sync.dma_start(out=xt[:], in_=xs[hb, ab])
                Xt = Xp.tile([KP, 4, XMF], bf16, tag="X", name=f"X{hg}{ab}")
                # host pre-split w into even/odd planes for unit-stride reads
                xv = xt[:].rearrange("k (c t e w2) -> k c t e w2",
                                     c=5, t=5, e=2)
                Xv = Xt[:].rearrange("k m (c t p) -> k m c t p", c=5, t=5)
                ev0 = xv[:, :, :, 0, 0:48]   # d0 = x[2p-1] (padded idx 2p)
                ev1 = xv[:, :, :, 0, 1:49]   # d2 = x[2p+1]
                od0 = xv[:, :, :, 1, 0:48]   # d1 = x[2p]
                od1 = xv[:, :, :, 1, 1:49]   # d3 = x[2p+2]
                nc.vector.tensor_sub(Xv[:, 0], ev0, ev1)   # m0 = d0 - d2
                nc.vector.tensor_add(Xv[:, 1], od0, ev1)   # m1 = d1 + d2
                nc.vector.tensor_sub(Xv[:, 2], ev1, od0)   # m2 = d2 - d1
                nc.gpsimd.tensor_sub(Xv[:, 3], od0, od1)   # m3 = d1 - d3
                return Xt

            def mm_set(ps, Xa, Xb, a, wh):
                ij, m = divmod(a, 4)
                i, j = divmod(ij, 3)
                sta = st[:, a * MP:(a + 1) * MP]
                rhs_a = Xa[:].rearrange("k m (c t p) -> k m c t p",
                                        c=5, t=5)[
                    :, m, i:i + 3, j:j + 3, wh * 24:wh * 24 + 24]
                rhs_b = Xb[:].rearrange("k m (c t p) -> k m c t p",
                                        c=5, t=5)[
                    :, m, i:i + 3, j:j + 3, wh * 24:wh * 24 + 24]
                # start=True clears has_written for the WHOLE bank, so only
                # the first matmul touching each bank (m=0 -> bank0,
                # m=2 -> bank1) may use it; the other m-region of the bank
                # starts from cleared bits (overwrite) without a new clear.
                nc.tensor.matmul(ps[0:MP, m, 0:PFREE], sta, rhs_a,
                                 start=(a in (0, 2)), stop=(ij == 8),
                                 skip_group_check=True)
                nc.tensor.matmul(ps[MP:2 * MP, m, 0:PFREE], sta, rhs_b,
                                 start=(a in (0, 2)), stop=(ij == 8),
                                 skip_group_check=True)

            def combine(ps, hb, wh):
                # DVE may read at most one PSUM operand per instruction;
                # the bias-seeded copies run on the idle scalar engine.
                tA = tp.tile([2 * MP, PFREE], f32, tag="tmp", name="tA")
                tB = tp.tile([2 * MP, PFREE], f32, tag="tmp", name="tB")
                ot = op.tile([2 * MP, 2, PFREE], f32, tag="ot", name="ot")
                # out0 = ((M0 + bias) + M1) + M2
                nc.scalar.activation(tA[:], ps[:, 0, 0:PFREE],
                                     mybir.ActivationFunctionType.Identity,
                                     bias=bt[:])
                nc.vector.tensor_add(tA[:], tA[:], ps[:, 1, 0:PFREE])
                nc.vector.tensor_add(ot[:, 0], tA[:], ps[:, 2, 0:PFREE])
                # out1 = ((-M3 + bias) + M1) - M2   (m=3 stationary negated)
                nc.scalar.activation(tB[:], ps[:, 3, 0:PFREE],
                                     mybir.ActivationFunctionType.Identity,
                                     bias=bt[:])
                nc.vector.tensor_add(tB[:], tB[:], ps[:, 1, 0:PFREE])
                nc.vector.tensor_sub(ot[:, 1], tB[:], ps[:, 2, 0:PFREE])
                nc.sync.dma_start(out=out[hb, wh], in_=ot[:])

            for g in range(HB // G):
                tiles = []
                for hg in range(G):
                    hb = g * G + hg
                    Xa = load_transform(hb, 0, hg)
                    Xb = load_transform(hb, 1, hg)
                    tiles.append((hb, Xa, Xb))
                pst = [[pp.tile([2 * MP, 4, 256], f32, tag="ps",
                                name=f"ps{hg}{wh}")
                        for wh in range(WH)] for hg in range(G)]
                if g == 0:
                    # set-major warmup: first matmuls only need the first
                    # x tile pair instead of the whole group
                    for hg, (hb, Xa, Xb) in enumerate(tiles):
                        for wh in range(WH):
                            for a in range(NW):
                                mm_set(pst[hg][wh], Xa, Xb, a, wh)
                            combine(pst[hg][wh], hb, wh)
                else:
                    for a in range(NW):
                        for hg, (hb, Xa, Xb) in enumerate(tiles):
                            for wh in range(WH):
                                mm_set(pst[hg][wh], Xa, Xb, a, wh)
                    for hg, (hb, Xa, Xb) in enumerate(tiles):
                        for wh in range(WH):
                            combine(pst[hg][wh], hb, wh)

    n_removed = _dedup_ldweights(nc, mybir)
    assert n_removed > 1000, f"ldweights dedup removed only {n_removed}"
    nc.compile()
    return nc


def _build_direct():
    import concourse.bacc as bacc
    import concourse.mybir as mybir
    from concourse.tile import TileContext

    bf16 = mybir.dt.bfloat16
    f32 = mybir.dt.float32

    nc = bacc.Bacc("TRN2", target_bir_lowering=False, debug=False,
                   num_devices=NCORES)
    xs = nc.dram_tensor("xs", [HB, 2, KP, XF], bf16,
                        kind="ExternalInput").ap()
    # stationaries pre-tiled [KP, NACC*MP] so one contiguous DMA loads all
    stat = nc.dram_tensor("stat", [KP, NACC * MP], bf16,
                          kind="ExternalInput").ap()
    bias = nc.dram_tensor("bias", [2 * MP, FREE], f32,
                          kind="ExternalInput").ap()
    out = nc.dram_tensor("out", [HB, WH, 2 * MP, FREE], f32,
                         kind="ExternalOutput").ap()

    ijkw = [(i, j, kw)
            for i in range(3) for j in range(3) for kw in range(3)]

    G = 3
    with TileContext(nc) as tc:
        with (tc.tile_pool(name="const", bufs=1) as cp,
              tc.tile_pool(name="xt", bufs=12) as xp,
              tc.tile_pool(name="ps", bufs=8, space="PSUM") as pp,
              tc.tile_pool(name="ot", bufs=4) as op):
            st = cp.tile([KP, NACC * MP], bf16)
            nc.sync.dma_start(out=st[:], in_=stat[:])
            bt = cp.tile([2 * MP, FREE], f32)
            nc.sync.dma_start(out=bt[:], in_=bias[:])

            for g in range(HB // G):
                tiles = []
                for hg in range(G):
                    hb = g * G + hg
                    ta = xp.tile([KP, XF], bf16, tag="x", name=f"xa{hg}")
                    tb = xp.tile([KP, XF], bf16, tag="x", name=f"xb{hg}")
                    nc.sync.dma_start(out=ta[:], in_=xs[hb, 0])
                    nc.sync.dma_start(out=tb[:], in_=xs[hb, 1])
                    tiles.append((hb, ta, tb))
                pst = [[pp.tile([2 * MP, FREE], f32, tag="ps",
                                name=f"ps{hg}{wh}")
                        for wh in range(WH)] for hg in range(G)]

                def mm_set(ps, ta, tb, a, wh):
                    i, j, kw = ijkw[a]
                    sta = st[:, a * MP:(a + 1) * MP]
                    base = wh * 48
                    rhs_a = ta[:].rearrange(
                        "k (c t w) -> k c t w", c=5, t=5)[
                        :, i:i + 3, j:j + 3, base + kw:base + kw + 48]
                    rhs_b = tb[:].rearrange(
                        "k (c t w) -> k c t w", c=5, t=5)[
                        :, i:i + 3, j:j + 3, base + kw:base + kw + 48]
                    nc.tensor.matmul(ps[0:MP, :], sta, rhs_a,
                                     start=(a == 0), stop=(a == NACC - 1))
                    nc.tensor.matmul(ps[MP:2 * MP, :], sta, rhs_b,
                                     start=(a == 0), stop=(a == NACC - 1))

                def drain(ps, hb, wh):
                    ot = op.tile([2 * MP, FREE], f32, tag="ot", name="ot")
                    nc.vector.tensor_add(ot[:], ps[:], bt[:])
                    nc.sync.dma_start(out=out[hb, wh], in_=ot[:])

                if g == 0:
                    # set-major warmup: first matmuls only need the first
                    # x tile pair instead of the whole group's DMAs
                    for hg, (hb, ta, tb) in enumerate(tiles):
                        for wh in range(WH):
                            for a in range(NACC):
                                mm_set(pst[hg][wh], ta, tb, a, wh)
                            drain(pst[hg][wh], hb, wh)
                else:
                    for a in range(NACC):
                        for hg, (hb, ta, tb) in enumerate(tiles):
                            for wh in range(WH):
                                mm_set(pst[hg][wh], ta, tb, a, wh)
                    for hg, (hb, ta, tb) in enumerate(tiles):
                        for wh in range(WH):
                            drain(pst[hg][wh], hb, wh)

    n_removed = _dedup_ldweights(nc, mybir)
    assert n_removed > 1000, f"ldweights dedup removed only {n_removed}"
    nc.compile()
    return nc


def _band_stat(vals):
    """Scatter per-(kd,kh) [ci,o] values into the banded stationary.
    vals[kd][kh] -> [ci, o]; returns [KP, MP] = [(slot,ci,dp), (o,d,hh)]."""
    S = np.zeros((4, C, D, O, D, 2), np.float32)
    for slot in range(4):
        for hh in range(2):
            kh = slot - hh
            if not 0 <= kh <= 2:
                continue
            for d in range(D):
                for kd in range(3):
                    dp = d + kd - 1
                    if not 0 <= dp < D:
                        continue
                    S[slot, :, dp, :, d, hh] = vals[kd][kh]
    return S.reshape(KP, MP)


def _host_prep(x, Wk, b, mode):
    import ml_dtypes
    npdt = ml_dtypes.bfloat16

    mean_b = (b.sum(0) / 9.0).astype(np.float32)

    if mode == "wino":
        Gm = np.array([[1, 0, 0], [.5, .5, .5], [.5, -.5, .5], [0, 0, 1]],
                      np.float64)
        S = np.zeros((NW, KP, MP), np.float32)
        for i in range(3):
            for j in range(3):
                ij = i * 3 + j
                # Gg[m, o, ci, kd, kh]
                Gg = np.einsum('mw,ockhw->mockh', Gm,
                               Wk[ij].astype(np.float64) / 9.0)
                Gg[3] = -Gg[3]   # negated so the combine is all adds
                for m in range(4):
                    vals = [[Gg[m, :, :, kd, kh].T.astype(np.float32)
                             for kh in range(3)] for kd in range(3)]
                    S[ij * 4 + m] = _band_stat(vals)
        S = S.astype(npdt)
        bias_arr = np.empty((2 * MP, 1), np.float32)
        for p in range(2 * MP):
            bias_arr[p, 0] = mean_b[(p % MP) // 16]
    else:
        S = np.zeros((NACC, KP, MP), np.float32)
        for i in range(3):
            for j in range(3):
                for kw in range(3):
                    a = (i * 3 + j) * 3 + kw
                    vals = [[Wk[i * 3 + j, :, :, kd, kh, kw].T / 9.0
                             for kh in range(3)] for kd in range(3)]
                    S[a] = _band_stat(vals)
        S = np.ascontiguousarray(
            S.transpose(1, 0, 2)).reshape(KP, NACC * MP).astype(npdt)
        bias_arr = np.empty((2 * MP, FREE), np.float32)
        for p in range(2 * MP):
            bias_arr[p, :] = mean_b[(p % MP) // 16]

    in_maps = []
    for core in range(NCORES):
        bb, cg, tg = core // 4, (core // 2) % 2, core % 2
        xsh = x[bb, :, cg * 3:cg * 3 + 5, tg * 3:tg * 3 + 5]  # [4,5,5,8,96,96]
        xpad = np.zeros((C, 5, 5, D, H + 2, WD + 2), np.float32)
        xpad[:, :, :, :, 1:H + 1, 1:WD + 1] = xsh
        # xs[hb, ab, (slot,ci,d), (cdim,td,w)]; window rows h'=4hb-1+2ab+slot
        xt = np.empty((HB, 2, 4, C, D, 5, 5, WD + 2), np.float32)
        for hb in range(HB):
            for ab in range(2):
                h0 = 4 * hb + 2 * ab
                blk = xpad[:, :, :, :, h0:h0 + 4, :]
                xt[hb, ab] = blk.transpose(4, 0, 3, 1, 2, 5)
        if mode == "wino":
            # split w into (even, odd) planes: [..., 98] -> [..., 2, 49]
            xt = np.ascontiguousarray(
                xt.reshape(HB, 2, 4, C, D, 5, 5, 49, 2).transpose(
                    0, 1, 2, 3, 4, 5, 6, 8, 7))
        in_maps.append({
            "xs": xt.reshape(HB, 2, KP, XF).astype(npdt),
            "stat": S,
            "bias": bias_arr,
        })
    return in_maps


def kernel(x, W, b, trace=False):
    x = np.asarray(x, np.float32)
    W = np.asarray(W, np.float32)
    b = np.asarray(b, np.float32)

    mode = MODE
    if mode not in _CACHE:
        _install_ntff_hook()
        _CACHE[mode] = _build_wino() if mode == "wino" else _build_direct()
    nc = _CACHE[mode]

    from concourse.bass_utils import run_bass_kernel_spmd
    in_maps = _host_prep(x, W, b, mode)
    res = run_bass_kernel_spmd(nc, in_maps, core_ids=list(range(NCORES)),
                               trace=trace)
    kernel.last_exec_ns = res.exec_time_ns

    outf = np.empty((B, O, CC, TT, D, H, WD), np.float32)
    for core in range(NCORES):
        bb, cg, tg = core // 4, (core // 2) % 2, core % 2
        r = res.results[core]["out"]  # [HB, WH, 128, 432]
        if mode == "wino":
            r = r.reshape(HB, WH, 2, O, D, 2, 2, 3, 3, 24)
            # (hb, wh, hp, o, d, hh, e, c, t, p)
            #  -> (o, c, t, d, hb, hp, hh, wh, p, e)
            r = r.transpose(3, 7, 8, 4, 0, 2, 5, 1, 9, 6)
            r = r.reshape(O, 3, 3, D, H, WD)
        else:
            r = r.reshape(HB, WH, 2, O, D, 2, 3, 3, 48)
            # (hb, wh, hp, o, d, hh, c, t, wc)
            r = r.transpose(3, 6, 7, 4, 0, 2, 5, 1, 8)
            r = r.reshape(O, 3, 3, D, H, WD)
        outf[bb, :, cg * 3:cg * 3 + 3, tg * 3:tg * 3 + 3] = r
    return outf


kernel.last_exec_ns = None
